# revision 32
# baseline (speedup 1.0000x reference)
"""MoE-routing (squeeze-excitation with K=4 conv1x1 experts) — Trainium2 Bass kernel.

Reference computation (per batch b):
    y    = mean_{h,w}(x[b])                      # [C]
    wk   = softmax(y @ fc_w.T + fc_b)            # [K]
    hid  = relu(W1 @ x[b] + b1)                  # [K*D, HW]  (all K experts stacked; K*D == 128)
    out  = (wk-scaled W2) @ hid + (wk @ b2)      # [C, HW]
    res  = x[b] * sigmoid(out)

Sharding: data-parallel over batch B=16 across 8 cores (2 batches per core),
all params replicated (tiny).

The whole data path runs in bf16 (inputs converted host-side): x is DMA'd in
as bf16 and the result written back as bf16, halving HBM traffic to
~16.8 MB/core (DMA roofline at 360 GB/s ~= 47 us).  The 2e-2 rel-err budget
dwarfs bf16's ~0.3% error.  bf16 also makes every matmul 1 cyc/row on the PE
(4x over f32) and unlocks the DVE 2x/4x perf modes.

Softmax stays UN-normalized on the critical path: e_k = exp(r_k) is folded
into w2 (w2p = e-scaled w2t) and the 1/sum(e) normalizer is applied for free
by the sigmoid's per-partition `scale` operand (out = sigmoid(in*scale+bias)).
exp(r) = (1+tanh(r/2))/(1-tanh(r/2)) so every ACT op (tanh / sigmoid) lives
in the single "sigmoid_and_others" table set -> one table load.

Engine budget per core (TimelineSim cost model, ~56.5 us total):
  ACT  : 32 sigmoids x [128,1024] from 2-bank PSUM groups (33.2 us) +
         3 early conv1-relus in its pre-stream idle window  <- critical
  DMA  : 8.4 MB in + 8.4 MB out + ~0.9 MB weights ~= 48 us of 360 GB/s
  DVE  : scans (tensor_scalar+accum, 4x mode) + late relus (PSUM->bf16) +
         ~2/3 of the final muls (2x mode) ~= 32 us
  Pool : ~1/3 of the final muls (GPSIMD tensor_mul, SBUF-only) ~= 22 us
  PE   : conv1 + conv2 bf16 (1 cyc/row) + routing matmuls ~= 28 us
PSUM: one pool of 4 x [128,1024] f32 slots (8 banks).  conv1 groups, conv2
groups and two routing-accumulator slots per batch all cycle through the
same ring; every conv2 allocation waits on a sigmoid ~4 periods back, which
gives each conv1-relu / routing producer ~4 us of slack.  Batch-1's routing
is emitted early so its tanh and tiny matmuls interleave into batch-0's
sigmoid stream with no drain at the transition.
"""

import numpy as np
import ml_dtypes

import concourse.bass as bass
import concourse.bacc as bacc
import concourse.mybir as mybir
import concourse.tile as tile
from concourse.bass_utils import run_bass_kernel_spmd

N_CORES = 8
B, C, H, W = 16, 512, 64, 64
HW = H * W                  # 4096
K, D = 4, 32
KD = K * D                  # 128 == partition count
P = 128
BPC = B // N_CORES          # batches per core = 2
NCH = C // P                # channel chunks = 4
TT = 512                    # matmul moving free-dim (one PSUM bank, fp32)
GT = 1024                   # PSUM group columns (2 banks)
TPG = GT // TT              # t-tiles per group = 2
NG = HW // GT               # groups per (batch, chunk) = 4
NQ = 4                      # x in-DMA waves per batch (quarters)
XQ = HW // NQ               # 1024

F32 = mybir.dt.float32
BF16 = mybir.dt.bfloat16
AF = mybir.ActivationFunctionType
ALU = mybir.AluOpType

# f32 side-blob layouts (columns)
WS_COLS = 19     # fcwt(16) + b1(1) + fcb/2(1, rows 0:4) + ones4(1, rows 0:4)
WR_COLS = 768    # emat [0:4,0:128], ones1x128 [0:1,128:256], b2t [0:4,256:768]


def build_bass():
    nc = bacc.Bacc("TRN2", target_bir_lowering=False)

    xs = nc.dram_tensor("xs", [BPC, C, HW], BF16, kind="ExternalInput")
    wb16 = nc.dram_tensor("wb16", [P, 1024], BF16, kind="ExternalInput")
    ws = nc.dram_tensor("ws", [P, WS_COLS], F32, kind="ExternalInput")
    wr = nc.dram_tensor("wr", [P, WR_COLS], F32, kind="ExternalInput")
    res = nc.dram_tensor("res", [BPC, C, HW], BF16, kind="ExternalOutput")

    with tile.TileContext(nc) as tc:
        with (
            tc.tile_pool(name="persist", bufs=1) as pp,
            tc.tile_pool(name="sig", bufs=6) as sgp,
            tc.tile_pool(name="pg", bufs=4, space="PSUM") as pgp,
        ):
            # ---- persistent SBUF tiles ----
            wb16_sb = pp.tile([P, 1024], BF16, tag="wb16")
            ws_sb = pp.tile([P, WS_COLS], F32, tag="ws")
            wr_sb = pp.tile([P, WR_COLS], F32, tag="wr")
            w1t_sb = wb16_sb[:, 0:512].rearrange("p (j m) -> p j m", j=NCH)
            w2t_sb = wb16_sb[:, 512:1024]
            fcwt_sb = ws_sb[:, 0:16].rearrange("p (j k) -> p j k", j=NCH)
            b1v_sb = ws_sb[:, 16:17]
            fcbh_sb = ws_sb[0:K, 17:18]
            ones4_sb = ws_sb[0:K, 18:19]
            emat_sb = wr_sb[0:K, 0:128]
            ones128_sb = wr_sb[0:1, 128:256]
            b2t_sb = wr_sb[0:K, 256:768]

            scr = pp.tile([P, 4], F32, tag="scr")
            dump = pp.tile([P, XQ], BF16, tag="dump")
            # routing accumulators ride a pg-pool slot per batch ("rt era"):
            # sub-ranges of one [P, GT] PSUM tile allocated between the
            # conv1 and conv2 eras of the ring
            rta, rtb = {}, {}

            def new_rt(b):
                # two slots so the w2p leg (fc/wv/folds) and the scale/bias
                # leg (b2/s/bc) carry no cross false deps (dep tracking is
                # tile-granular)
                rta[b] = pgp.tile([P, GT], F32, tag="pg", name=f"rta{b}")
                rtb[b] = pgp.tile([P, GT], F32, tag="pg", name=f"rtb{b}")

            def r_acc(b):
                return rta[b][0:K, 0:1]

            def wv_t(b):
                return rta[b][:, 64:65]

            def warm_t(b):
                return rta[b][:, 256:320]

            def s_t(b):
                return rtb[b][0:1, 0:1]

            def bc_t(b):
                return rtb[b][:, 64:65]

            def b2_t(b):
                return rtb[b][:, 128:128 + NCH]

            xt = {}
            hid = {}
            ysum = {}
            for b in range(BPC):
                for j in range(NCH):
                    xt[b, j] = pp.tile([P, HW], BF16, tag=f"x{b}{j}",
                                       name=f"x{b}{j}")
                hid[b] = pp.tile([KD, HW], BF16, tag=f"hid{b}", name=f"hid{b}")
                for j in range(NCH):
                    for q in range(NQ):
                        ysum[b, j, q] = pp.tile(
                            [P, 1], F32, tag=f"ys{b}{j}{q}",
                            name=f"ys{b}{j}{q}")

            # Force the single ACT table-set load (tanh -> sigmoid_and_others)
            # while the ACT queue is empty.
            nc.vector.memset(scr[0:1, 0:1], 0.0)
            nc.scalar.activation(out=scr[0:1, 1:2], in_=scr[0:1, 0:1],
                                 func=AF.Sigmoid)
            nc.scalar.activation(out=scr[0:1, 2:3], in_=scr[0:1, 0:1],
                                 func=AF.Tanh)

            # ---- all DMAs, in arrival-priority order ----
            nc.sync.dma_start(out=wb16_sb[:, 0:512], in_=wb16[:, 0:512])
            for q in range(NQ):
                for j in range(NCH):
                    nc.sync.dma_start(
                        out=xt[0, j][:, q * XQ:(q + 1) * XQ],
                        in_=xs[0, j * P:(j + 1) * P, q * XQ:(q + 1) * XQ])
                if q == 0:
                    nc.sync.dma_start(out=ws_sb, in_=ws[:, :])
                if q == 1:
                    nc.sync.dma_start(out=wb16_sb[:, 512:1024],
                                      in_=wb16[:, 512:1024])
            nc.sync.dma_start(out=wr_sb[:, 0:256], in_=wr[:, 0:256])
            nc.sync.dma_start(out=wr_sb[:, 256:768], in_=wr[:, 256:768])
            for q in range(NQ):
                for j in range(NCH):
                    nc.sync.dma_start(
                        out=xt[1, j][:, q * XQ:(q + 1) * XQ],
                        in_=xs[1, j * P:(j + 1) * P, q * XQ:(q + 1) * XQ])

            # ---- per-instruction emitters ----
            scan_i = {}

            def scan1(b, j, q):
                # row-sum of one loaded x quarter-chunk (DVE 4x mode)
                scan_i[b, j, q] = nc.vector.tensor_scalar(
                    out=dump, in0=xt[b, j][:, q * XQ:(q + 1) * XQ],
                    scalar1=1.0, scalar2=0.0, op0=ALU.mult, op1=ALU.add,
                    accum_out=ysum[b, j, q])

            def scan(b, q, js=range(NCH)):
                for j in js:
                    scan1(b, j, q)

            p1g = {}

            def pass1_mm(b, g):
                # conv1 matmuls for group g (t-tiles 2g, 2g+1) -> PSUM group
                p1g[b, g] = pgp.tile([KD, GT], F32, tag="pg", name=f"p1g{b}{g}")
                for ti in range(TPG):
                    t = g * TPG + ti
                    for j in range(NCH):
                        nc.tensor.matmul(
                            p1g[b, g][:, ti * TT:(ti + 1) * TT],
                            lhsT=w1t_sb[:, j, :],
                            rhs=xt[b, j][:, t * TT:(t + 1) * TT],
                            start=(j == 0), stop=(j == NCH - 1),
                            skip_group_check=True)

            def pass1_relu_act(b, g):
                # same as pass1_relu but on the (idle, pre-sigmoid) ACT
                nc.scalar.activation(
                    out=hid[b][:, g * GT:(g + 1) * GT], in_=p1g[b, g],
                    func=AF.Relu, bias=b1v_sb)

            def pass1_relu(b, g, after=None):
                # hid = max(conv1 + b1, 0), PSUM f32 -> SBUF bf16 (DVE).
                # `after` pins a scheduling-order dep (no semaphore) to keep
                # the 1.2us relu out of latency-critical DVE windows.
                i = nc.vector.tensor_scalar(
                    out=hid[b][:, g * GT:(g + 1) * GT], in0=p1g[b, g],
                    scalar1=b1v_sb, scalar2=0.0, op0=ALU.add, op1=ALU.max)
                if after is not None:
                    tile.add_dep_helper(i.ins, after.ins, sync=False,
                                        reason="relu ordering")
                return i

            def fc_mm(b, cols, start, stop):
                # routing partials: r += fcwt.T @ ysum[:, col], accumulated in
                # the persistent bank at rps[0:K, 0:1]
                for i, col in enumerate(cols):
                    j, q = col // NQ, col % NQ
                    nc.tensor.matmul(
                        r_acc(b), lhsT=fcwt_sb[:, j, :],
                        rhs=ysum[b, j, q],
                        start=(start and i == 0),
                        stop=(stop and i == len(cols) - 1),
                        skip_group_check=True)

            warm_c = [0]

            def pe_warm(b, n):
                # tiny throwaway matmuls into a spare range of the routing
                # slot: keep the PE p-state ramped across data-arrival gaps
                for _ in range(n):
                    warm_c[0] += 1
                    nc.tensor.matmul(
                        warm_t(b), lhsT=w2t_sb[:, 0:P],
                        rhs=w2t_sb[:, 0:64], start=True, stop=True,
                        skip_group_check=True)

            w2p, srec128, bias2 = {}, {}, {}

            def routing_head(b):
                # r (PSUM) -> e = exp(r+fcb) via tanh -> w2p = e-scaled w2t
                th = pp.tile([K, 1], F32, tag=f"th{b}", name=f"th{b}")
                nc.scalar.activation(out=th, in_=r_acc(b), func=AF.Tanh,
                                     bias=fcbh_sb, scale=0.5)
                onep = pp.tile([K, 1], F32, tag=f"op{b}", name=f"op{b}")
                nc.vector.tensor_scalar(out=onep, in0=th, scalar1=1.0,
                                        scalar2=None, op0=ALU.add)
                onem = pp.tile([K, 1], F32, tag=f"om{b}", name=f"om{b}")
                nc.vector.tensor_scalar(out=onem, in0=th, scalar1=-1.0,
                                        scalar2=1.0, op0=ALU.mult, op1=ALU.add)
                onem_r = pp.tile([K, 1], F32, tag=f"omr{b}", name=f"omr{b}")
                nc.vector.reciprocal(out=onem_r, in_=onem)
                e_sb = pp.tile([K, 1], F32, tag=f"e{b}", name=f"e{b}")
                nc.vector.tensor_mul(e_sb, onep, onem_r)
                # bias leg first into rtb: b2 then s (their only real dep
                # is e); then the broadcast of 1/s
                for cj in range(NCH):
                    nc.tensor.matmul(
                        b2_t(b)[:, cj:cj + 1],
                        lhsT=b2t_sb[:, cj * P:(cj + 1) * P], rhs=e_sb,
                        start=True, stop=True, skip_group_check=True)
                nc.tensor.matmul(s_t(b), lhsT=e_sb, rhs=ones4_sb,
                                 start=True, stop=True, skip_group_check=True)
                srec = pp.tile([1, 1], F32, tag=f"sr{b}", name=f"sr{b}")
                nc.vector.reciprocal(out=srec, in_=s_t(b))
                # w2p leg: wvec = expand e over (k,d) partitions, fold into w2
                nc.tensor.matmul(wv_t(b), lhsT=emat_sb, rhs=e_sb,
                                 start=True, stop=True, skip_group_check=True)
                bci = nc.tensor.matmul(bc_t(b), lhsT=ones128_sb, rhs=srec,
                                       start=True, stop=True,
                                       skip_group_check=True)
                i = None
                for cj in range(NCH):
                    w2p[b, cj] = pp.tile([P, P], BF16, tag=f"w2p{b}{cj}",
                                         name=f"w2p{b}{cj}")
                    i = nc.vector.tensor_scalar_mul(
                        w2p[b, cj], w2t_sb[:, cj * P:(cj + 1) * P],
                        wv_t(b))
                return e_sb, i, bci

            def routing_tail(b, e_sb):
                # bias2 = (e @ b2) * (1/sum e); srec128 = broadcast 1/sum(e)
                srec128[b] = pp.tile([P, 1], F32, tag=f"s128{b}",
                                     name=f"s128{b}")
                nc.scalar.copy(srec128[b], bc_t(b))
                # bias2 = b2_t * srec128, on ACT (Copy with scale AP) so the
                # whole bias leg stays on one engine right before sigmoid 0
                bias2[b] = pp.tile([P, NCH], F32, tag=f"bias2{b}",
                                   name=f"bias2{b}")
                return nc.scalar.activation(out=bias2[b], in_=b2_t(b),
                                            func=AF.Copy,
                                            scale=srec128[b][:, 0:1])

            def pass2_mm(b, cj, g, after=None):
                # conv2 matmuls for output group (b, cj, cols g*GT...)
                og = pgp.tile([P, GT], F32, tag="pg", name=f"o{b}{cj}{g}")
                for ti in range(TPG):
                    t = g * TPG + ti
                    i = nc.tensor.matmul(
                        og[:, ti * TT:(ti + 1) * TT],
                        lhsT=w2p[b, cj],
                        rhs=hid[b][:, t * TT:(t + 1) * TT],
                        start=True, stop=True)
                    if after is not None:
                        tile.add_dep_helper(i.ins, after.ins, sync=False,
                                            reason="routing tinies first")
                        after = None
                return og

            def pass2_fin(b, cj, g, og, c0=0, c1=GT, after=None):
                # attn = sigmoid(out * (1/sum e) + bias2): one ACT op; then
                # res = x * attn in place over x (DVE 2x mode, or offloaded
                # to the otherwise-idle GPSIMD for cj==1), then DMA out
                sg = sgp.tile([P, GT], BF16, tag="sig", name="sg")
                nc.scalar.activation(out=sg[:, c0:c1], in_=og[:, c0:c1],
                                     func=AF.Sigmoid,
                                     scale=srec128[b][:, 0:1],
                                     bias=bias2[b][:, cj:cj + 1])
                xsl = xt[b, cj][:, g * GT + c0:g * GT + c1]
                on_pool = cj in (1, 3) if b == 0 else (cj == 1 and g < 2)
                eng = nc.gpsimd if on_pool else nc.vector
                i = eng.tensor_mul(xsl, sg[:, c0:c1], xsl)
                if after is not None:
                    tile.add_dep_helper(i.ins, after.ins, sync=False,
                                        reason="mul after b1 scans")
                nc.sync.dma_start(
                    out=res[b, cj * P:(cj + 1) * P, g * GT + c0:g * GT + c1],
                    in_=xsl)

            p2q = []

            def og_emit(b, cj, g, after=None):
                p2q.append((b, cj, g, pass2_mm(b, cj, g, after=after)))

            def og_drain(after=None):
                b, cj, g, og = p2q.pop(0)
                pass2_fin(b, cj, g, og, after=after)

            # ================= schedule =================
            # Ring plan (bufs=4, slots cycle A,B,C,D by allocation order):
            # conv1 groups are consumed by relus that finish early (the
            # first three on the pre-sigmoid-idle ACT), the routing "rt"
            # slot is allocated before the fc matmuls need it, and every
            # conv2 (og) allocation waits a sigmoid ~4 periods back.
            scan(0, 0)
            pass1_mm(0, 0)
            scan(0, 1)
            pass1_mm(0, 1)
            pass1_relu_act(0, 0)
            scan(0, 2)
            pass1_mm(0, 2)
            pass1_relu_act(0, 1)
            new_rt(0)
            fc_mm(0, [j * NQ + q for j in range(NCH) for q in range(2)],
                  start=True, stop=False)
            fc_mm(0, [j * NQ + 2 for j in range(NCH)], start=False, stop=False)
            pe_warm(0, 3)
            pass1_relu_act(0, 2)
            scan(0, 3)
            fc_mm(0, [j * NQ + 3 for j in range(NCH)], start=False, stop=True)
            # -- routing b0: critical chain to the first sigmoid
            e0, w2pi0, bci0 = routing_head(0)
            b2i0 = routing_tail(0, e0)
            og_emit(0, 0, 0)
            og_emit(0, 1, 0)
            og_emit(0, 2, 0)
            og_emit(0, 3, 0)
            og_drain()                  # (0,0,0)
            pass1_mm(0, 3)
            og_drain()                  # (0,1,0)
            og_emit(0, 0, 1)
            pass1_relu(0, 3)
            og_drain()                  # (0,2,0)
            og_emit(0, 1, 1)
            scan(1, 0, js=(0, 1))
            og_drain()                  # (0,3,0)
            og_emit(0, 2, 1)
            scan(1, 0, js=(2, 3))
            og_drain()                  # (0,0,1)
            og_emit(0, 3, 1)
            og_drain()                  # (0,1,1)
            og_emit(0, 0, 2)
            pass1_mm(1, 0)
            og_drain()                  # (0,2,1)
            og_emit(0, 1, 2)
            scan(1, 1, js=(0, 1))
            og_drain()                  # (0,3,1)
            og_emit(0, 2, 2)
            scan(1, 1, js=(2, 3))
            og_drain()                  # (0,0,2)
            og_emit(0, 3, 2)
            pass1_relu(1, 0)
            og_drain()                  # (0,1,2)
            og_emit(0, 0, 3)
            scan(1, 2)
            og_drain()                  # (0,2,2)
            og_emit(0, 1, 3)
            pass1_mm(1, 1)
            og_drain()                  # (0,3,2)
            og_emit(0, 2, 3)
            scan(1, 3)
            og_drain()                  # (0,0,3)
            # -- batch-1 routing: emitted early so its tanh and tiny
            #    matmuls slot into the batch-0 sigmoid stream; the rt slot
            #    is allocated where the ring frees it by ~29us
            new_rt(1)
            fc_mm(1, list(range(NCH * NQ)), start=True, stop=True)
            e1, w2pi1, bci1 = routing_head(1)
            b2i1 = routing_tail(1, e1)
            og_emit(0, 3, 3)
            pass1_relu(1, 1)
            og_drain()                  # (0,1,3)
            og_drain()                  # (0,2,3)
            og_emit(1, 0, 0)
            og_drain()                  # (0,3,3)
            og_emit(1, 1, 0)
            og_drain()                  # (1,0,0)
            og_emit(1, 2, 0)
            pass1_mm(1, 2)
            og_drain()                  # (1,1,0)
            og_emit(1, 3, 0)
            pass1_relu(1, 2)
            og_drain()                  # (1,2,0)
            og_emit(1, 0, 1)
            pass1_mm(1, 3)
            og_drain()                  # (1,3,0)
            og_emit(1, 1, 1)
            pass1_relu(1, 3)
            og_drain()                  # (1,0,1)
            og_emit(1, 2, 1)
            og_drain()                  # (1,1,1)
            og_emit(1, 3, 1)
            og_drain()                  # (1,2,1)
            rest = [(cj, g) for g in range(2, NG) for cj in range(NCH)]
            for cj, g in rest:
                og_emit(1, cj, g)
                og_drain()
            # final group: drain in two pieces to pipeline the kernel tail
            b, cj, g, og = p2q.pop(0)
            pass2_fin(b, cj, g, og, 0, 768)
            pass2_fin(b, cj, g, og, 768, GT)

    nc.compile()
    return nc


_NC_CACHE = None


def _get_nc():
    global _NC_CACHE
    if _NC_CACHE is None:
        _NC_CACHE = build_bass()
    return _NC_CACHE


def _prep_inputs(x, fc_w, fc_b, w1, b1, w2, b2):
    """Host-side weight re-layouts + per-core shards (bf16 data path)."""
    f = np.float32
    bf = ml_dtypes.bfloat16
    x = np.ascontiguousarray(x, dtype=f).reshape(B, C, HW).astype(bf)
    wb16 = np.zeros((P, 1024), dtype=bf)
    # w1t[p, j*128+m] = w1[k(m), d(m), j*128+p]  (lhsT chunks for conv1)
    w1t = w1.transpose(2, 0, 1).reshape(C, KD)          # [c, kd]
    wb16[:, 0:512] = w1t.reshape(NCH, P, KD).transpose(1, 0, 2).reshape(
        P, 512).astype(bf)
    wb16[:, 512:1024] = w2.transpose(0, 2, 1).reshape(KD, C).astype(bf)
    wsb = np.zeros((P, WS_COLS), dtype=f)
    fcwt = (fc_w.T / HW).astype(f)                      # [c, k]
    wsb[:, 0:16] = fcwt.reshape(NCH, P, K).transpose(1, 0, 2).reshape(P, 16)
    wsb[:, 16] = b1.reshape(KD)
    wsb[0:K, 17] = fc_b / 2.0
    wsb[0:K, 18] = 1.0
    wrb = np.zeros((P, WR_COLS), dtype=f)
    wrb[0:K, 0:128] = np.kron(np.eye(K), np.ones((1, D)))
    wrb[0, 128:256] = 1.0
    wrb[0:K, 256:768] = b2
    shared = {"wb16": wb16, "ws": wsb, "wr": wrb}
    in_maps = []
    for i in range(N_CORES):
        m = dict(shared)
        m["xs"] = np.ascontiguousarray(x[i * BPC:(i + 1) * BPC])
        in_maps.append(m)
    return in_maps


def run_on_device(inputs, trace=False):
    """Returns (full_output [B,C,H,W] f32, BassKernelResults)."""
    nc = _get_nc()
    in_maps = _prep_inputs(**inputs)
    r = run_bass_kernel_spmd(
        nc, in_maps, core_ids=list(range(N_CORES)), trace=trace
    )
    out = np.concatenate([np.asarray(d["res"]) for d in r.results], axis=0)
    return out.reshape(B, C, H, W).astype(np.float32), r


def kernel(**inputs) -> np.ndarray:
    out, _ = run_on_device(inputs, trace=False)
    return out
